# revision 39
# baseline (speedup 1.0000x reference)
"""Multi-head causal attention block (qkv -> softmax(QK^T/sqrt(d)+mask) V -> proj)
on 8 Trainium2 NeuronCores.

Sharding: 8 cores = 4 batches (data parallel) x 2 head-groups of 8 heads
(tensor parallel: W_qkv column-sharded, W_proj row-sharded). Each core
computes a partial projection output for its (batch, head-group); the host
sums the two partials per batch (the "all-reduce") and adds b_proj.

Core kernel (per core, all matmuls fp16 with fp32 psum accumulate):
  - qT/kT computed in [d, n] layout, v in [n, d] layout (x pre-transposed on
    host so every matmul contracts over the partition dim).
  - attention uses transposed scores S^T[k, q] = (kT_tile).T @ qT so that the
    softmax denominator comes for free from a ones-column augmented V
    (out[64] = column sums) and P^T never needs an on-chip transpose.
  - causal structure: fully-masked 128x128 blocks are skipped; on diagonal
    blocks the mask is applied as a post-exp multiply by host-precomputed
    exp(mask) (exp(s+m) = exp(s)*exp(m)), avoiding any PSUM read-modify-write.
  - exp on ScalarE without max subtraction (logits are O(5) here; exact for
    the softmax up to fp rounding).
  - the PE stream is kept dense: qkv for group g+1 and the projection for
    group g-1 are emitted as "filler" psum-tile tasks between attention
    head-pairs, so the tensor engine never idles waiting for exp/normalize
    and holds its top p-state clock.
"""

import numpy as np

B, N, C = 4, 2048, 1024
H, D = 16, 64
G = 2                  # head groups (cores = B * G = 8)
HPC = H // G           # heads per core
DG = HPC * D           # 512 = per-core qkv width per projection
NT = N // 128          # 16 k/n tiles
QG = N // 512          # 4 q groups
VW = 65                # v_aug width per head (ones col + 64 dims)

_CACHE = {}


def _classify_blocks(attn_mask):
    """Per 128x128 block (j=k-tile, i=q-tile): 0 all-zero, 1 all-masked, 2 mixed."""
    sub = np.empty((NT, NT), dtype=np.int8)
    for j in range(NT):
        for i in range(NT):
            blk = attn_mask[i * 128:(i + 1) * 128, j * 128:(j + 1) * 128]
            if np.all(blk == 0.0):
                sub[j, i] = 0
            elif np.all(blk <= -150.0):
                sub[j, i] = 1
            else:
                sub[j, i] = 2
    return sub


def _build_plan(attn_mask):
    """Plan: for each (qgroup i4, k-tile j) either skip or compute cols
    [lo,hi) (128-units within the 512-wide group) with optional mask add
    (segment id, add_lo, add_hi). Returns plan + concatenated mask segments."""
    sub = _classify_blocks(attn_mask)
    segs = {}
    seg_list = []
    plan = []  # list over i4 of list of (j, lo, hi, mseg or None)
    for i4 in range(QG):
        entries = []
        for j in range(NT):
            states = [sub[j, 4 * i4 + qc] for qc in range(4)]
            keep = [qc for qc in range(4) if states[qc] != 1]
            if not keep:
                continue
            lo, hi = min(keep), max(keep) + 1
            need = [qc for qc in range(lo, hi) if states[qc] != 0]
            mseg = None
            if need:
                alo, ahi = min(need), max(need) + 1
                i0 = (4 * i4 + alo) * 128
                i1 = (4 * i4 + ahi) * 128
                seg = np.exp(np.ascontiguousarray(
                    attn_mask[i0:i1, j * 128:(j + 1) * 128].T).astype(
                        np.float64)).astype(np.float32)
                key = (ahi - alo, seg.tobytes())
                if key not in segs:
                    segs[key] = sum(s.shape[1] // 128 for s in seg_list)
                    seg_list.append(seg)
                mseg = (segs[key], alo, ahi)
            entries.append((j, lo, hi, mseg))
        plan.append(entries)
    if seg_list:
        masks_np = np.concatenate(seg_list, axis=1)
    else:
        masks_np = np.zeros((128, 128), dtype=np.float32)
    return plan, masks_np


def _build_program(plan, mask_width):
    import concourse.mybir as mybir
    import concourse.tile as tile
    from concourse import bacc

    F32 = mybir.dt.float32
    F16 = mybir.dt.float16
    AF = mybir.ActivationFunctionType

    nc = bacc.Bacc("TRN2", target_bir_lowering=False, debug=False, num_devices=8)
    xT = nc.dram_tensor("xT", [C, N], F16, kind="ExternalInput").ap()
    wqkv = nc.dram_tensor("wqkv", [C, 3 * DG], F16, kind="ExternalInput").ap()
    wp = nc.dram_tensor("wp", [DG, C], F16, kind="ExternalInput").ap()
    masks = nc.dram_tensor("masks", [128, mask_width], F16, kind="ExternalInput").ap()
    ones = nc.dram_tensor("ones", [128, 128], F16, kind="ExternalInput").ap()
    out = nc.dram_tensor("out", [N, C], F32, kind="ExternalOutput").ap()

    with tile.TileContext(nc) as tc:
        with (tc.tile_pool(name="pers", bufs=1) as pers,
              tc.tile_pool(name="psS", bufs=2, space="PSUM") as psS,
              tc.tile_pool(name="psV", bufs=1, space="PSUM") as psV,
              tc.tile_pool(name="psF", bufs=1, space="PSUM") as psF,
              tc.tile_pool(name="xp", bufs=4) as xp,
              tc.tile_pool(name="ep", bufs=6) as ep,
              tc.tile_pool(name="aop", bufs=4) as aop,
              tc.tile_pool(name="nrm", bufs=2) as nrm,
              tc.tile_pool(name="op", bufs=3) as op):
            # fp16 q/k/v: input-rounding error only (~5e-4), exact matmuls
            sq = pers.tile([128, 4 * N], F16, tag="sq")
            sk = pers.tile([128, 4 * N], F16, tag="sk")
            sv = pers.tile([128, NT * HPC * VW], F16, tag="sv")
            smask = pers.tile([128, mask_width], F16, tag="smask")
            sones = pers.tile([128, 128], F16, tag="sones")
            swq = pers.tile([128, 8 * 3 * DG], F16, tag="swq")
            swp = pers.tile([128, 4 * C], F16, tag="swp")

            xts = {}

            def dma_x(g, chunked):
                xt = xp.tile([128, 8 * 512], F16, tag="x", name=f"x{g}")
                src = xT.rearrange("(kt p) n -> p kt n", p=128)
                if chunked:
                    for kt in range(8):
                        nc.sync.dma_start(
                            xt[:, kt * 512:kt * 512 + 512],
                            src[:, kt, g * 512:(g + 1) * 512])
                else:
                    nc.sync.dma_start(
                        xt[:].rearrange("p (kt n) -> p kt n", kt=8),
                        src[:, :, g * 512:(g + 1) * 512])
                xts[g] = xt

            # startup DMAs: per-kt q-weight/x/k-weight triplets so the first
            # qkv matmul starts after ~1 chunk and the first scores follow
            # the k projection as early as possible
            wsrc = wqkv.rearrange("(kt p) c -> p kt c", p=128)
            xts[0] = xp.tile([128, 8 * 512], F16, tag="x", name="x0")
            xsrc = xT.rearrange("(kt p) n -> p kt n", p=128)
            for kt in range(8):
                nc.sync.dma_start(
                    swq[:, kt * 1536:kt * 1536 + 512], wsrc[:, kt, 0:512])
                nc.sync.dma_start(
                    xts[0][:, kt * 512:kt * 512 + 512], xsrc[:, kt, 0:512])
                nc.sync.dma_start(
                    swq[:, kt * 1536 + 512:kt * 1536 + 1024],
                    wsrc[:, kt, 512:1024])
            nc.sync.dma_start(smask[:], masks)
            nc.sync.dma_start(sones[:], ones)
            for kt in range(8):
                nc.sync.dma_start(
                    swq[:, kt * 1536 + 1024:kt * 1536 + 1536],
                    wsrc[:, kt, 1024:1536])
            nc.sync.dma_start(
                swp[:].rearrange("p (kt c) -> p kt c", kt=4),
                wp.rearrange("(kt p) c -> p kt c", p=128))
            # ones column (at index 64) for every (n-tile, head)
            nc.vector.tensor_copy(
                sv[:].rearrange("p (t c) -> p t c", c=VW)[:, :, 64:65],
                sones[:])

            # ------------- PE tile tasks (qkv pieces / proj pieces) ---------
            # each task is a generator yielding once per matmul so the
            # emission can be spread ("pumped") between attention entries
            fill_alt = [0]

            def fill_pool(fonly):
                fill_alt[0] ^= 1
                return psF if fill_alt[0] else psS

            def task_qk(g, proj, mp, fonly=False):
                pool = fill_pool(fonly)
                ps = pool.tile([128, 1024], F32, tag="sc")
                xt = xts[g]
                for sub in range(2):
                    mt = mp * 2 + sub
                    for kt in range(8):
                        if sub or kt:
                            yield
                        nc.tensor.matmul(
                            ps[:, sub * 512:sub * 512 + 512],
                            swq[:, kt * 1536 + proj * DG + mt * 128:
                                kt * 1536 + proj * DG + mt * 128 + 128],
                            xt[:, kt * 512:kt * 512 + 512],
                            start=(kt == 0), stop=(kt == 7))
                dst = sq if proj == 0 else sk
                nc.vector.tensor_copy(
                    dst[:].rearrange("p (mt n) -> p mt n", n=N)
                      [:, mp * 2:mp * 2 + 2, g * 512:g * 512 + 512],
                    ps[:].rearrange("p (s n) -> p s n", n=512))

            def task_v(g, sp, fonly=False):
                pool = fill_pool(fonly)
                ps = pool.tile([128, 1024], F32, tag="sc")
                xt = xts[g]
                for sub in range(2):
                    tok = (sp * 2 + sub) * 128
                    for kt in range(8):
                        if sub or kt:
                            yield
                        nc.tensor.matmul(
                            ps[:, sub * 512:sub * 512 + 512],
                            xt[:, kt * 512 + tok:kt * 512 + tok + 128],
                            swq[:, kt * 1536 + 1024:kt * 1536 + 1536],
                            start=(kt == 0), stop=(kt == 7))
                nt_i = g * 4 + sp * 2
                nc.vector.tensor_copy(
                    sv[:].rearrange("p (t h c) -> p t h c", h=HPC, c=VW)
                      [:, nt_i:nt_i + 2, :, 0:D],
                    ps[:].rearrange("p (s h c) -> p s h c", s=2, c=D))

            attns = {}

            def task_proj(i4, sub, fonly=False):
                pool = fill_pool(fonly)
                ps = pool.tile([128, 1024], F32, tag="sc")
                attn = attns[i4]
                for fg in range(2):
                    for ct in range(4):
                        if fg or ct:
                            yield
                        nc.tensor.matmul(
                            ps[:, fg * 512:fg * 512 + 512],
                            attn[:, ct * 512 + sub * 128:
                                 ct * 512 + sub * 128 + 128],
                            swp[:, ct * C + fg * 512:ct * C + fg * 512 + 512],
                            start=(ct == 0), stop=(ct == 3))
                ot = op.tile([128, 1024], F32, tag="out")
                nc.vector.tensor_copy(ot[:], ps[:])
                nc.sync.dma_start(
                    out[i4 * 512 + sub * 128:i4 * 512 + sub * 128 + 128, :],
                    ot[:])

            def run_now(gen):
                for _ in gen:
                    pass


            # head-pair units are processed in an interleaved order that
            # sandwiches the exp-heaviest group-3 units between lighter ones
            # so the scalar engine is never locally the bottleneck; fillers
            # (keyed by unit index) pad each unit's PE work up to its exp time
            UNIT_ORDER = [(0, 0), (0, 1), (0, 2), (0, 3), (1, 0), (1, 1),
                          (3, 0), (1, 2), (1, 3), (3, 1), (2, 0), (3, 2),
                          (2, 1), (3, 3), (2, 2), (2, 3)]
            SCHED = {
                0: [("qk", 0, 0, 1)],
                1: [("qk", 0, 1, 1)],
                2: [("qk", 1, 0, 0), ("qk", 1, 1, 0)],
                3: [("v", 1, 0), ("v", 1, 1)],
                4: [("qk", 2, 1, 0), ("qk", 3, 0, 0), ("qk", 3, 1, 0)],
                5: [("v", 2, 0), ("v", 2, 1), ("v", 3, 0), ("v", 3, 1)],
                6: [("qk", 1, 0, 1), ("qk", 1, 1, 1)],
                7: [("qk", 2, 0, 0), ("proj", 0, 0)],
                8: [("qk", 3, 0, 1), ("proj", 0, 1)],
                9: [("qk", 3, 1, 1), ("qk", 2, 1, 1)],
                10: [("proj", 0, 2), ("proj", 1, 0)],
                11: [("proj", 0, 3), ("proj", 1, 1), ("proj", 1, 2)],
                12: [("qk", 2, 0, 1), ("proj", 1, 3)],
                13: [("proj", 3, 0), ("proj", 3, 1), ("proj", 3, 2)],
                14: [("proj", 3, 3)],
                15: [],
            }

            def make_task(t, fonly=False):
                if t[0] == "qk":
                    return task_qk(t[1], t[2], t[3], fonly)
                elif t[0] == "v":
                    return task_v(t[1], t[2], fonly)
                return task_proj(t[1], t[2], fonly)

            # -------- attention for one q-group --------
            def emit_unit(i4, hm, uidx):
                if hm == 0:
                    attns[i4] = aop.tile([128, 4 * 512], F16, tag="attn",
                                         name=f"attn{i4}")
                attn = attns[i4]
                entries = plan[i4]
                first_j = entries[0][0]
                last_j = entries[-1][0]
                # head pair: head h0 at PE rows 0-63, h0+1 at rows 64-127;
                # adjacent scores matmuls hit different row groups and run
                # concurrently. PV trails scores by 2 entries (software
                # pipeline) so it rarely stalls the PE waiting on exp.
                if True:
                    h0 = 2 * hm
                    ppv = psV.tile([VW, 1024], F32, tag="pv")

                    def emit_pv(j, l0, et):
                        for hh in range(2):
                            nc.tensor.matmul(
                                ppv[:, hh * 512 + l0:hh * 512 + 512],
                                sv[:, (j * HPC + h0 + hh) * VW:
                                   (j * HPC + h0 + hh) * VW + VW],
                                et[:, hh * 512 + l0:hh * 512 + 512],
                                start=(j == first_j), stop=(j == last_j))

                    pending = []
                    for (j, lo, hi, mseg) in entries:
                        l0 = lo * 128
                        pscr = psS.tile([128, 1024], F32, tag="sc")
                        for hh in range(2):
                            hp = hh * 64
                            nc.tensor.matmul(
                                pscr[:, hh * 512 + l0:hh * 512 + 512],
                                sk[hp:hp + 64,
                                   hm * N + j * 128:hm * N + j * 128 + 128],
                                sq[hp:hp + 64, hm * N + i4 * 512 + l0:
                                   hm * N + i4 * 512 + 512],
                                start=True, stop=True)
                        et = ep.tile([128, 1024], F16, tag="exp")
                        if l0 == 0:
                            nc.scalar.activation(et[:], pscr[:], AF.Exp)
                        else:
                            # two exps skip the inter-head garbage columns
                            nc.scalar.activation(et[:, l0:512],
                                                 pscr[:, l0:512], AF.Exp)
                            nc.scalar.activation(et[:, 512 + l0:1024],
                                                 pscr[:, 512 + l0:1024], AF.Exp)
                        if mseg is not None:
                            soff, alo, ahi = mseg
                            w = (ahi - alo) * 128
                            for hh in range(2):
                                nc.vector.tensor_mul(
                                    et[:, hh * 512 + alo * 128:
                                       hh * 512 + alo * 128 + w],
                                    et[:, hh * 512 + alo * 128:
                                       hh * 512 + alo * 128 + w],
                                    smask[:, soff * 128:soff * 128 + w])
                        pending.append((j, l0, et))
                        if len(pending) > 2:
                            emit_pv(*pending.pop(0))
                    while pending:
                        emit_pv(*pending.pop(0))
                    # normalize: rows 0..63 / row 64 (the ones-column sums)
                    srow = nrm.tile([1, 1024], F32, tag="srow")
                    rec = nrm.tile([1, 1024], F32, tag="rec")
                    scr = nrm.tile([1, 1024], F32, tag="scr")
                    bc = nrm.tile([64, 1024], F32, tag="bc")
                    nc.vector.tensor_copy(srow[:], ppv[64:65, :])
                    nc.vector.reciprocal_approx_accurate(
                        rec[:], srow[:], scr[:])
                    nc.gpsimd.partition_broadcast(bc[:, 0:512], rec[:, 0:512])
                    nc.gpsimd.partition_broadcast(bc[:, 512:1024], rec[:, 512:1024])
                    nc.vector.tensor_mul(
                        attn[0:64, hm * 512:hm * 512 + 512],
                        ppv[0:64, 0:512], bc[:, 0:512])
                    nc.vector.tensor_mul(
                        attn[64:128, hm * 512:hm * 512 + 512],
                        ppv[0:64, 512:1024], bc[:, 512:1024])
                    for t in SCHED.get(uidx, []):
                        run_now(make_task(t))

            # ---------------- main schedule ----------------
            # startup: minimal qkv(0) prefix inline, then attention groups
            # with fillers keeping the PE dense.
            run_now(task_qk(0, 0, 0))
            run_now(task_qk(0, 1, 0))
            run_now(task_v(0, 0))
            run_now(task_v(0, 1))
            dma_x(1, chunked=False)
            dma_x(2, chunked=False)
            dma_x(3, chunked=False)
            for uidx, (g, hm) in enumerate(UNIT_ORDER):
                emit_unit(g, hm, uidx)
            for sub in range(4):
                run_now(task_proj(2, sub))
    nc.compile()
    return nc


def _get_program(attn_mask):
    key = attn_mask.tobytes()
    if key not in _CACHE:
        plan, masks_np = _build_plan(attn_mask)
        nc = _build_program(plan, masks_np.shape[1])
        _CACHE[key] = (nc, masks_np)
    return _CACHE[key]


def _make_in_maps(x, attn_mask, W_qkv, W_proj, masks_np):
    w4 = W_qkv.reshape(C, 3, H, D)
    ones = np.ones((128, 128), dtype=np.float16)
    in_maps = []
    for core in range(8):
        b, g = core // G, core % G
        hs = slice(g * HPC, (g + 1) * HPC)
        wq = (w4[:, 0, hs, :] / np.sqrt(D)).reshape(C, DG)
        wk = w4[:, 1, hs, :].reshape(C, DG)
        wv = w4[:, 2, hs, :].reshape(C, DG)
        in_maps.append({
            "xT": np.ascontiguousarray(x[b].T).astype(np.float16),
            "wqkv": np.ascontiguousarray(
                np.concatenate([wq, wk, wv], axis=1)).astype(np.float16),
            "wp": np.ascontiguousarray(
                W_proj[g * DG:(g + 1) * DG, :]).astype(np.float16),
            "masks": masks_np.astype(np.float16),
            "ones": ones,
        })
    return in_maps


def kernel(x, attn_mask, W_qkv, W_proj, b_proj, **run_kwargs):
    from concourse import bass_utils

    x = np.asarray(x, dtype=np.float32)
    attn_mask = np.asarray(attn_mask, dtype=np.float32)
    W_qkv = np.asarray(W_qkv, dtype=np.float32)
    W_proj = np.asarray(W_proj, dtype=np.float32)
    b_proj = np.asarray(b_proj, dtype=np.float32)

    nc, masks_np = _get_program(attn_mask)
    in_maps = _make_in_maps(x, attn_mask, W_qkv, W_proj, masks_np)

    res = bass_utils.run_bass_kernel_spmd(nc, in_maps, core_ids=list(range(8)),
                                          **run_kwargs)
    outp = np.empty((B, N, C), dtype=np.float32)
    for b in range(B):
        outp[b] = res.results[2 * b]["out"] + res.results[2 * b + 1]["out"] + b_proj
    if run_kwargs:
        kernel.last_result = res
    return outp


# revision 40
# speedup vs baseline: 1.0489x; 1.0489x over previous
"""Multi-head causal attention block (qkv -> softmax(QK^T/sqrt(d)+mask) V -> proj)
on 8 Trainium2 NeuronCores.

Sharding: 8 cores = 4 batches (data parallel) x 2 head-groups of 8 heads
(tensor parallel: W_qkv column-sharded, W_proj row-sharded). Each core
computes a partial projection output for its (batch, head-group); the host
sums the two partials per batch (the "all-reduce") and adds b_proj.

Core kernel (per core, all matmuls fp16 with fp32 psum accumulate):
  - qT/kT computed in [d, n] layout, v in [n, d] layout (x pre-transposed on
    host so every matmul contracts over the partition dim).
  - attention uses transposed scores S^T[k, q] = (kT_tile).T @ qT so that the
    softmax denominator comes for free from a ones-column augmented V
    (out[64] = column sums) and P^T never needs an on-chip transpose.
  - causal structure: fully-masked 128x128 blocks are skipped; on diagonal
    blocks the mask is applied as a post-exp multiply by host-precomputed
    exp(mask) (exp(s+m) = exp(s)*exp(m)), avoiding any PSUM read-modify-write.
  - exp on ScalarE without max subtraction (logits are O(5) here; exact for
    the softmax up to fp rounding).
  - the PE stream is kept dense: qkv for group g+1 and the projection for
    group g-1 are emitted as "filler" psum-tile tasks between attention
    head-pairs, so the tensor engine never idles waiting for exp/normalize
    and holds its top p-state clock.
"""

import numpy as np

B, N, C = 4, 2048, 1024
H, D = 16, 64
G = 2                  # head groups (cores = B * G = 8)
HPC = H // G           # heads per core
DG = HPC * D           # 512 = per-core qkv width per projection
NT = N // 128          # 16 k/n tiles
QG = N // 512          # 4 q groups
VW = 65                # v_aug width per head (ones col + 64 dims)

_CACHE = {}


def _classify_blocks(attn_mask):
    """Per 128x128 block (j=k-tile, i=q-tile): 0 all-zero, 1 all-masked, 2 mixed."""
    sub = np.empty((NT, NT), dtype=np.int8)
    for j in range(NT):
        for i in range(NT):
            blk = attn_mask[i * 128:(i + 1) * 128, j * 128:(j + 1) * 128]
            if np.all(blk == 0.0):
                sub[j, i] = 0
            elif np.all(blk <= -150.0):
                sub[j, i] = 1
            else:
                sub[j, i] = 2
    return sub


def _build_plan(attn_mask):
    """Plan: for each (qgroup i4, k-tile j) either skip or compute cols
    [lo,hi) (128-units within the 512-wide group) with optional mask add
    (segment id, add_lo, add_hi). Returns plan + concatenated mask segments."""
    sub = _classify_blocks(attn_mask)
    segs = {}
    seg_list = []
    plan = []  # list over i4 of list of (j, lo, hi, mseg or None)
    for i4 in range(QG):
        entries = []
        for j in range(NT):
            states = [sub[j, 4 * i4 + qc] for qc in range(4)]
            keep = [qc for qc in range(4) if states[qc] != 1]
            if not keep:
                continue
            lo, hi = min(keep), max(keep) + 1
            need = [qc for qc in range(lo, hi) if states[qc] != 0]
            mseg = None
            if need:
                alo, ahi = min(need), max(need) + 1
                i0 = (4 * i4 + alo) * 128
                i1 = (4 * i4 + ahi) * 128
                seg = np.exp(np.ascontiguousarray(
                    attn_mask[i0:i1, j * 128:(j + 1) * 128].T).astype(
                        np.float64)).astype(np.float32)
                key = (ahi - alo, seg.tobytes())
                if key not in segs:
                    segs[key] = sum(s.shape[1] // 128 for s in seg_list)
                    seg_list.append(seg)
                mseg = (segs[key], alo, ahi)
            entries.append((j, lo, hi, mseg))
        plan.append(entries)
    if seg_list:
        masks_np = np.concatenate(seg_list, axis=1)
    else:
        masks_np = np.zeros((128, 128), dtype=np.float32)
    return plan, masks_np


def _build_program(plan, mask_width):
    import concourse.mybir as mybir
    import concourse.tile as tile
    from concourse import bacc

    F32 = mybir.dt.float32
    F16 = mybir.dt.float16
    AF = mybir.ActivationFunctionType

    nc = bacc.Bacc("TRN2", target_bir_lowering=False, debug=False, num_devices=8)
    xT = nc.dram_tensor("xT", [C, N], F16, kind="ExternalInput").ap()
    wqkv = nc.dram_tensor("wqkv", [C, 3 * DG], F16, kind="ExternalInput").ap()
    wp = nc.dram_tensor("wp", [DG, C], F16, kind="ExternalInput").ap()
    masks = nc.dram_tensor("masks", [128, mask_width], F16, kind="ExternalInput").ap()
    ones = nc.dram_tensor("ones", [128, 128], F16, kind="ExternalInput").ap()
    out = nc.dram_tensor("out", [N, C], F32, kind="ExternalOutput").ap()

    with tile.TileContext(nc) as tc:
        with (tc.tile_pool(name="pers", bufs=1) as pers,
              tc.tile_pool(name="psS", bufs=2, space="PSUM") as psS,
              tc.tile_pool(name="psV", bufs=1, space="PSUM") as psV,
              tc.tile_pool(name="psF", bufs=1, space="PSUM") as psF,
              tc.tile_pool(name="xp", bufs=3) as xp,
              tc.tile_pool(name="ep", bufs=6) as ep,
              tc.tile_pool(name="aop", bufs=3) as aop,
              tc.tile_pool(name="nrm", bufs=2) as nrm,
              tc.tile_pool(name="op", bufs=3) as op):
            # fp16 q/k/v: input-rounding error only (~5e-4), exact matmuls
            sq = pers.tile([128, 4 * N], F16, tag="sq")
            sk = pers.tile([128, 4 * N], F16, tag="sk")
            sv = pers.tile([128, NT * HPC * VW], F16, tag="sv")
            smask = pers.tile([128, mask_width], F16, tag="smask")
            sones = pers.tile([128, 128], F16, tag="sones")
            swq = pers.tile([128, 8 * 3 * DG], F16, tag="swq")
            swp = pers.tile([128, 4 * C], F16, tag="swp")

            xts = {}

            def dma_x(g, chunked):
                xt = xp.tile([128, 8 * 512], F16, tag="x", name=f"x{g}")
                src = xT.rearrange("(kt p) n -> p kt n", p=128)
                if chunked:
                    for kt in range(8):
                        nc.sync.dma_start(
                            xt[:, kt * 512:kt * 512 + 512],
                            src[:, kt, g * 512:(g + 1) * 512])
                else:
                    nc.sync.dma_start(
                        xt[:].rearrange("p (kt n) -> p kt n", kt=8),
                        src[:, :, g * 512:(g + 1) * 512])
                xts[g] = xt

            # startup DMAs: per-kt q-weight/x/k-weight triplets so the first
            # qkv matmul starts after ~1 chunk and the first scores follow
            # the k projection as early as possible
            wsrc = wqkv.rearrange("(kt p) c -> p kt c", p=128)
            xts[0] = xp.tile([128, 8 * 512], F16, tag="x", name="x0")
            xsrc = xT.rearrange("(kt p) n -> p kt n", p=128)
            for kt in range(8):
                nc.sync.dma_start(
                    swq[:, kt * 1536:kt * 1536 + 512], wsrc[:, kt, 0:512])
                nc.sync.dma_start(
                    xts[0][:, kt * 512:kt * 512 + 512], xsrc[:, kt, 0:512])
                nc.sync.dma_start(
                    swq[:, kt * 1536 + 512:kt * 1536 + 1024],
                    wsrc[:, kt, 512:1024])
            nc.sync.dma_start(smask[:], masks)
            nc.sync.dma_start(sones[:], ones)
            for kt in range(8):
                nc.sync.dma_start(
                    swq[:, kt * 1536 + 1024:kt * 1536 + 1536],
                    wsrc[:, kt, 1024:1536])
            nc.sync.dma_start(
                swp[:].rearrange("p (kt c) -> p kt c", kt=4),
                wp.rearrange("(kt p) c -> p kt c", p=128))
            # ones column (at index 64) for every (n-tile, head)
            nc.vector.tensor_copy(
                sv[:].rearrange("p (t c) -> p t c", c=VW)[:, :, 64:65],
                sones[:])

            # ------------- PE tile tasks (qkv pieces / proj pieces) ---------
            # each task is a generator yielding once per matmul so the
            # emission can be spread ("pumped") between attention entries
            fill_alt = [0]

            def fill_pool(fonly):
                fill_alt[0] ^= 1
                return psF if fill_alt[0] else psS

            def task_qk(g, proj, mp, fonly=False):
                pool = fill_pool(fonly)
                ps = pool.tile([128, 1024], F32, tag="sc")
                xt = xts[g]
                for sub in range(2):
                    mt = mp * 2 + sub
                    for kt in range(8):
                        if sub or kt:
                            yield
                        nc.tensor.matmul(
                            ps[:, sub * 512:sub * 512 + 512],
                            swq[:, kt * 1536 + proj * DG + mt * 128:
                                kt * 1536 + proj * DG + mt * 128 + 128],
                            xt[:, kt * 512:kt * 512 + 512],
                            start=(kt == 0), stop=(kt == 7))
                dst = sq if proj == 0 else sk
                nc.vector.tensor_copy(
                    dst[:].rearrange("p (mt n) -> p mt n", n=N)
                      [:, mp * 2:mp * 2 + 2, g * 512:g * 512 + 512],
                    ps[:].rearrange("p (s n) -> p s n", n=512))

            def task_v(g, sp, fonly=False):
                pool = fill_pool(fonly)
                ps = pool.tile([128, 1024], F32, tag="sc")
                xt = xts[g]
                for sub in range(2):
                    tok = (sp * 2 + sub) * 128
                    for kt in range(8):
                        if sub or kt:
                            yield
                        nc.tensor.matmul(
                            ps[:, sub * 512:sub * 512 + 512],
                            xt[:, kt * 512 + tok:kt * 512 + tok + 128],
                            swq[:, kt * 1536 + 1024:kt * 1536 + 1536],
                            start=(kt == 0), stop=(kt == 7))
                nt_i = g * 4 + sp * 2
                nc.vector.tensor_copy(
                    sv[:].rearrange("p (t h c) -> p t h c", h=HPC, c=VW)
                      [:, nt_i:nt_i + 2, :, 0:D],
                    ps[:].rearrange("p (s h c) -> p s h c", s=2, c=D))

            attns = {}

            def task_proj(i4, sub, fonly=False):
                pool = fill_pool(fonly)
                ps = pool.tile([128, 1024], F32, tag="sc")
                attn = attns[i4]
                for fg in range(2):
                    for ct in range(4):
                        if fg or ct:
                            yield
                        nc.tensor.matmul(
                            ps[:, fg * 512:fg * 512 + 512],
                            attn[:, ct * 512 + sub * 128:
                                 ct * 512 + sub * 128 + 128],
                            swp[:, ct * C + fg * 512:ct * C + fg * 512 + 512],
                            start=(ct == 0), stop=(ct == 3))
                ot = op.tile([128, 1024], F32, tag="out")
                nc.vector.tensor_copy(ot[:], ps[:])
                nc.sync.dma_start(
                    out[i4 * 512 + sub * 128:i4 * 512 + sub * 128 + 128, :],
                    ot[:])

            def run_now(gen):
                for _ in gen:
                    pass


            # filler schedule per (group, head-pair boundary); groups are
            # processed 0,1,3,2 so the exp-heaviest group (3) sits where the
            # most filler PE work is available, and the lighter group 2
            # absorbs the final projection work
            SCHED = {
                (0, 0): [("qk", 0, 0, 1), ("qk", 0, 1, 1)],
                (0, 2): [("qk", 1, 0, 0), ("qk", 1, 1, 0), ("v", 1, 0)],
                (0, 3): [("v", 1, 1)],
                (1, 0): [("qk", 1, 0, 1), ("qk", 1, 1, 1), ("qk", 2, 1, 0)],
                (1, 1): [("qk", 2, 1, 1)],
                (1, 2): [("qk", 3, 0, 0), ("qk", 3, 1, 0), ("v", 2, 0)],
                (1, 3): [("v", 3, 0), ("v", 3, 1), ("v", 2, 1)],
                (3, 0): [("qk", 3, 0, 1), ("qk", 3, 1, 1), ("proj", 0, 0)],
                (3, 1): [("proj", 0, 1)],
                (3, 2): [("qk", 2, 0, 0), ("proj", 0, 2), ("proj", 1, 0)],
                (3, 3): [("proj", 0, 3), ("proj", 1, 1), ("proj", 1, 2)],
                (2, 0): [("qk", 2, 0, 1), ("proj", 1, 3)],
                (2, 1): [("proj", 3, 0)],
                (2, 2): [("proj", 3, 1), ("proj", 3, 2)],
                (2, 3): [("proj", 3, 3)],
            }

            def make_task(t, fonly=False):
                if t[0] == "qk":
                    return task_qk(t[1], t[2], t[3], fonly)
                elif t[0] == "v":
                    return task_v(t[1], t[2], fonly)
                return task_proj(t[1], t[2], fonly)

            # -------- attention for one q-group --------
            def emit_attention(i4):
                attn = aop.tile([128, 4 * 512], F16, tag="attn",
                                name=f"attn{i4}")
                attns[i4] = attn
                entries = plan[i4]
                first_j = entries[0][0]
                last_j = entries[-1][0]
                # head pairs: head h0 at PE rows 0-63, h0+1 at rows 64-127;
                # adjacent scores matmuls hit different row groups and run
                # concurrently. PV trails scores by 2 entries (software
                # pipeline) so it rarely stalls the PE waiting on exp.
                for h0 in range(0, HPC, 2):
                    hm = h0 // 2
                    ppv = psV.tile([VW, 1024], F32, tag="pv")

                    def emit_pv(j, l0, et):
                        for hh in range(2):
                            nc.tensor.matmul(
                                ppv[:, hh * 512 + l0:hh * 512 + 512],
                                sv[:, (j * HPC + h0 + hh) * VW:
                                   (j * HPC + h0 + hh) * VW + VW],
                                et[:, hh * 512 + l0:hh * 512 + 512],
                                start=(j == first_j), stop=(j == last_j))

                    pending = []
                    for (j, lo, hi, mseg) in entries:
                        l0 = lo * 128
                        pscr = psS.tile([128, 1024], F32, tag="sc")
                        for hh in range(2):
                            hp = hh * 64
                            nc.tensor.matmul(
                                pscr[:, hh * 512 + l0:hh * 512 + 512],
                                sk[hp:hp + 64,
                                   hm * N + j * 128:hm * N + j * 128 + 128],
                                sq[hp:hp + 64, hm * N + i4 * 512 + l0:
                                   hm * N + i4 * 512 + 512],
                                start=True, stop=True)
                        et = ep.tile([128, 1024], F16, tag="exp")
                        if l0 == 0:
                            nc.scalar.activation(et[:], pscr[:], AF.Exp)
                        else:
                            # two exps skip the inter-head garbage columns
                            nc.scalar.activation(et[:, l0:512],
                                                 pscr[:, l0:512], AF.Exp)
                            nc.scalar.activation(et[:, 512 + l0:1024],
                                                 pscr[:, 512 + l0:1024], AF.Exp)
                        if mseg is not None:
                            soff, alo, ahi = mseg
                            w = (ahi - alo) * 128
                            for hh in range(2):
                                nc.vector.tensor_mul(
                                    et[:, hh * 512 + alo * 128:
                                       hh * 512 + alo * 128 + w],
                                    et[:, hh * 512 + alo * 128:
                                       hh * 512 + alo * 128 + w],
                                    smask[:, soff * 128:soff * 128 + w])
                        pending.append((j, l0, et))
                        if len(pending) > 2:
                            emit_pv(*pending.pop(0))
                    while pending:
                        emit_pv(*pending.pop(0))
                    # normalize: rows 0..63 / row 64 (the ones-column sums)
                    srow = nrm.tile([1, 1024], F32, tag="srow")
                    rec = nrm.tile([1, 1024], F32, tag="rec")
                    scr = nrm.tile([1, 1024], F32, tag="scr")
                    bc = nrm.tile([64, 1024], F32, tag="bc")
                    nc.vector.tensor_copy(srow[:], ppv[64:65, :])
                    nc.vector.reciprocal_approx_accurate(
                        rec[:], srow[:], scr[:])
                    nc.gpsimd.partition_broadcast(bc[:, 0:512], rec[:, 0:512])
                    nc.gpsimd.partition_broadcast(bc[:, 512:1024], rec[:, 512:1024])
                    nc.vector.tensor_mul(
                        attn[0:64, hm * 512:hm * 512 + 512],
                        ppv[0:64, 0:512], bc[:, 0:512])
                    nc.vector.tensor_mul(
                        attn[64:128, hm * 512:hm * 512 + 512],
                        ppv[0:64, 512:1024], bc[:, 512:1024])
                    for t in SCHED.get((i4, hm), []):
                        run_now(make_task(t))

            # ---------------- main schedule ----------------
            # startup: minimal qkv(0) prefix inline, then attention groups
            # with fillers keeping the PE dense.
            run_now(task_qk(0, 0, 0))
            run_now(task_qk(0, 1, 0))
            run_now(task_v(0, 0))
            run_now(task_v(0, 1))
            PRE_DMA = {0: [1], 1: [2, 3]}  # prefetch x ahead of its tasks
            for g in (0, 1, 3, 2):
                for nxt in PRE_DMA.get(g, []):
                    dma_x(nxt, chunked=False)
                emit_attention(g)
            for sub in range(4):
                run_now(task_proj(2, sub))
    nc.compile()
    return nc


def _get_program(attn_mask):
    key = attn_mask.tobytes()
    if key not in _CACHE:
        plan, masks_np = _build_plan(attn_mask)
        nc = _build_program(plan, masks_np.shape[1])
        _CACHE[key] = (nc, masks_np)
    return _CACHE[key]


def _make_in_maps(x, attn_mask, W_qkv, W_proj, masks_np):
    w4 = W_qkv.reshape(C, 3, H, D)
    ones = np.ones((128, 128), dtype=np.float16)
    in_maps = []
    for core in range(8):
        b, g = core // G, core % G
        hs = slice(g * HPC, (g + 1) * HPC)
        wq = (w4[:, 0, hs, :] / np.sqrt(D)).reshape(C, DG)
        wk = w4[:, 1, hs, :].reshape(C, DG)
        wv = w4[:, 2, hs, :].reshape(C, DG)
        in_maps.append({
            "xT": np.ascontiguousarray(x[b].T).astype(np.float16),
            "wqkv": np.ascontiguousarray(
                np.concatenate([wq, wk, wv], axis=1)).astype(np.float16),
            "wp": np.ascontiguousarray(
                W_proj[g * DG:(g + 1) * DG, :]).astype(np.float16),
            "masks": masks_np.astype(np.float16),
            "ones": ones,
        })
    return in_maps


def kernel(x, attn_mask, W_qkv, W_proj, b_proj, **run_kwargs):
    from concourse import bass_utils

    x = np.asarray(x, dtype=np.float32)
    attn_mask = np.asarray(attn_mask, dtype=np.float32)
    W_qkv = np.asarray(W_qkv, dtype=np.float32)
    W_proj = np.asarray(W_proj, dtype=np.float32)
    b_proj = np.asarray(b_proj, dtype=np.float32)

    nc, masks_np = _get_program(attn_mask)
    in_maps = _make_in_maps(x, attn_mask, W_qkv, W_proj, masks_np)

    res = bass_utils.run_bass_kernel_spmd(nc, in_maps, core_ids=list(range(8)),
                                          **run_kwargs)
    outp = np.empty((B, N, C), dtype=np.float32)
    for b in range(B):
        outp[b] = res.results[2 * b]["out"] + res.results[2 * b + 1]["out"] + b_proj
    if run_kwargs:
        kernel.last_result = res
    return outp
